# revision 33
# baseline (speedup 1.0000x reference)
"""Dot-product attention (B=8, S=2048, DK=DV=512) on 8 TRN2 NeuronCores.

Data-parallel: one batch element per core. Per core, in transposed-score
layout (so the exp'ed attention chunks are directly the stationary operand
of the second matmul — no attention transposes needed):

    scores^T[k, q] = K @ Q^T / sqrt(DK)      (PE, fp32r, 1 col/cycle)
    attn^T = exp(scores^T + mask_bias[k])    (ACT, fused scale+bias+exp)
    den[q] = sum_k attn^T[k, q]              (DVE chunk-accumulate ->
                                              PE transpose -> DVE reduce;
                                              NO tiny-dim matmuls: M=1/N=1
                                              matmuls cost ~10us each on HW)
    out[q, v] = (attn^T).T @ V / den[q]      (PE accumulate + DVE normalize)

Only Q and K need physical transposes ([d, seq] layout for the contraction);
done on the PE via identity-matmul transpose, pipelined with the DMA loads.
scores are ~N(0,1) so softmax without max-subtraction is numerically safe;
masked keys get an additive -1e4 bias folded into the exp (exp -> 0).
"""

import os
from contextlib import ExitStack

import numpy as np

import concourse.mybir as mybir
import concourse.tile as tile
from concourse import bacc
from concourse.bass_utils import run_bass_kernel_spmd
from concourse.masks import make_identity

B = 8
S = 2048
DK = 512
DV = 512
P = 128

F32 = mybir.dt.float32
MASK_BIAS = -10000.0

# matmul compute dtype: float32r streams 1 row/cycle (vs 4 for float32) with
# fp32 storage and relaxed multiply precision; producers must round to it.
# Set BASS_ATTN_MM_DT=float32 (exact, 4x slower) or bfloat16 to change.
MM_DT = getattr(mybir.dt, os.environ.get("BASS_ATTN_MM_DT", "float32r"))


def build_attention(
    nc,
    s=S,
    dk=DK,
    dv=DV,
    q_tile=512,
    n_reps=1,
    use_t=True,
    use_exp=True,
    use_den=True,
    use_mm2=True,
):
    """Emit the per-core attention kernel into `nc` (TileContext inside).

    n_reps > 1 repeats the whole computation (for benchmarking: one NEFF
    timing K serialized executions; output is overwritten each rep).
    use_* flags disable pieces for timing bisection (output garbage).
    """
    scale = 1.0 / float(np.sqrt(dk))
    nkc = s // P          # key chunks (partition dim of scores^T)
    ndc = dk // P         # contraction chunks for QK^T
    nqt = s // q_tile     # outer q tiles
    nqs = q_tile // P     # q subtiles per q tile
    nst = s // P          # seq tiles for the Q/K transpose prep

    qd = nc.declare_dram_parameter("Q", [s, dk], F32, isOutput=False).ap()
    kd = nc.declare_dram_parameter("K", [s, dk], F32, isOutput=False).ap()
    vd = nc.declare_dram_parameter("V", [s, dv], F32, isOutput=False).ap()
    mbd = nc.declare_dram_parameter("MB", [P, nkc], F32, isOutput=False).ap()
    od = nc.declare_dram_parameter("out", [s, dv], F32, isOutput=True).ap()

    with ExitStack() as ctx:
        tc = ctx.enter_context(tile.TileContext(nc))
        constp = ctx.enter_context(tc.tile_pool(name="const", bufs=1))
        qtp = ctx.enter_context(tc.tile_pool(name="qtp", bufs=nqt + 1))
        ktp = ctx.enter_context(tc.tile_pool(name="ktp", bufs=nst + 2))
        vp = ctx.enter_context(tc.tile_pool(name="vp", bufs=nkc + 2))
        stagep = ctx.enter_context(tc.tile_pool(name="stage", bufs=8))
        attp = ctx.enter_context(tc.tile_pool(name="att", bufs=nkc + 4))
        osbp = ctx.enter_context(tc.tile_pool(name="osb", bufs=3))
        dsbp = ctx.enter_context(tc.tile_pool(name="dsb", bufs=2))
        tpp = ctx.enter_context(tc.tile_pool(name="tpp", bufs=2, space="PSUM"))
        stp = ctx.enter_context(tc.tile_pool(name="stp", bufs=2, space="PSUM"))
        opp = ctx.enter_context(tc.tile_pool(name="opp", bufs=2, space="PSUM"))
        dentp = ctx.enter_context(tc.tile_pool(name="dentp", bufs=2, space="PSUM"))

        id_sb = constp.tile([P, P], F32)
        make_identity(nc, id_sb[:])
        id_r = None
        if MM_DT == mybir.dt.float32r:
            id_r = constp.tile([P, P], MM_DT)
            nc.vector.tensor_copy(id_r[:], id_sb[:])
        mb_sb = constp.tile([P, nkc], F32)
        nc.sync.dma_start(mb_sb[:], mbd[:, :])

        for _rep in range(n_reps):
            # Transposed operands, fine-grained so DMA/transpose pipeline
            # into the matmuls:
            #   qt_q[i]  = Q^T for q tile i   [P, ndc, q_tile]
            #   kt_t[t]  = K^T for key chunk t [P, ndc, P]
            #   v_t[kc]  = natural V chunk     [P, dv]
            qt_q = [qtp.tile([P, ndc, q_tile], MM_DT, name=f"qtq{i}", tag="qtq")
                    for i in range(nqt)]
            kt_t = [ktp.tile([P, ndc, P], MM_DT, name=f"ktt{t}", tag="ktt")
                    for t in range(nst)]
            v_t = [vp.tile([P, dv], MM_DT, name=f"vt{kc}", tag="vt")
                   for kc in range(nkc)]

            tr_dt = (
                MM_DT
                if MM_DT == mybir.dt.float32r
                and os.environ.get("BASS_ATTN_TR_DT", "f32") == "f32r"
                else F32
            )

            def load_transpose(src, t, dst):
                nat = stagep.tile([P, dk], F32, tag="nat", name="nat")
                nc.sync.dma_start(nat[:], src[t * P : (t + 1) * P, :])
                if tr_dt == F32:
                    natr = nat
                else:
                    # round to fp32r on DVE so the PE transpose (a matmul)
                    # runs at 1.5 cycles/row instead of f32's 2.0
                    natr = stagep.tile([P, dk], tr_dt, tag="natr", name="natr")
                    nc.vector.tensor_copy(natr[:], nat[:])
                tp = tpp.tile([P, ndc * P], tr_dt, name="tp", tag="tp")
                for dc in range(ndc):
                    nc.tensor.transpose(
                        tp[:, dc * P : (dc + 1) * P],
                        natr[:, dc * P : (dc + 1) * P],
                        id_sb[:] if tr_dt == F32 else id_r[:],
                    )
                nc.vector.tensor_copy(
                    dst, tp[:].rearrange("p (dc q) -> p dc q", dc=ndc)
                )

            def load_v(kc):
                if MM_DT == F32:
                    nc.sync.dma_start(v_t[kc][:], vd[kc * P : (kc + 1) * P, :])
                else:
                    vn = stagep.tile([P, dv], F32, tag="vn", name="vn")
                    nc.sync.dma_start(vn[:], vd[kc * P : (kc + 1) * P, :])
                    nc.vector.tensor_copy(v_t[kc][:], vn[:])

            # ---- phase 1: emit in consumption order: K chunk 0-3 and Q
            # tile 0 first (unblocks the first matmuls), then the rest.
            if use_t:
                sq = q_tile // P
                for t in range(sq):
                    load_transpose(kd, t, kt_t[t][:, :, :])
                for t in range(sq):
                    load_transpose(qd, t, qt_q[0][:, :, t * P : (t + 1) * P])
                for t in range(sq, nst):
                    load_transpose(kd, t, kt_t[t][:, :, :])
                    load_transpose(
                        qd, t, qt_q[t // sq][:, :, (t % sq) * P : (t % sq + 1) * P]
                    )
            for kc in range(nkc):
                load_v(kc)

            # ---- phase 2: attention over q tiles ----
            for qt_i in range(nqt):
                q0 = qt_i * q_tile
                at_tiles = []
                acc = dsbp.tile([P, q_tile], F32, tag="acc")
                for kc in range(nkc):
                    st = stp.tile([P, q_tile], F32)
                    for dc in range(ndc):
                        nc.tensor.matmul(
                            st[:],
                            kt_t[kc][:, dc, :],
                            qt_q[qt_i][:, dc, :],
                            start=(dc == 0),
                            stop=(dc == ndc - 1),
                        )
                    at = attp.tile([P, q_tile], MM_DT, tag="at")
                    nc.scalar.activation(
                        at[:],
                        st[:],
                        mybir.ActivationFunctionType.Exp
                        if use_exp
                        else mybir.ActivationFunctionType.Copy,
                        bias=mb_sb[:, kc : kc + 1] if use_exp else 0.0,
                        scale=scale,
                    )
                    at_tiles.append(at)
                    if use_den:
                        # running sum of attn chunks over key chunks (DVE);
                        # fp32r holds f32 bits, read it as f32 for the add
                        atf = (
                            at[:].bitcast(F32)
                            if MM_DT == mybir.dt.float32r
                            else at[:]
                        )
                        if kc == 0:
                            nc.vector.tensor_copy(acc[:], atf)
                        else:
                            nc.vector.tensor_add(acc[:], acc[:], atf)

                # den[q] = partition-sum of acc: PE-transpose the 4 q-slices
                # into one PSUM bank, one 3D free-dim reduce, one reciprocal.
                if use_den:
                    dent_ps = dentp.tile([P, nqs * P], F32)
                    for qs in range(nqs):
                        nc.tensor.transpose(
                            dent_ps[:, qs * P : (qs + 1) * P],
                            acc[:, qs * P : (qs + 1) * P],
                            id_sb[:],
                        )
                    den_sb = dsbp.tile([P, nqs], F32, tag="den")
                    nc.vector.reduce_sum(
                        den_sb[:],
                        dent_ps[:].rearrange("p (qs k) -> p qs k", qs=nqs),
                        axis=mybir.AxisListType.X,
                    )
                    recip_sb = dsbp.tile([P, nqs], F32, tag="recip")
                    nc.vector.reciprocal(recip_sb[:], den_sb[:])

                for qs in range(nqs):
                    ob = osbp.tile([P, dv], F32, tag="ob")
                    if use_mm2:
                        op = opp.tile([P, dv], F32)
                        for kc in range(nkc):
                            nc.tensor.matmul(
                                op[:],
                                at_tiles[kc][:, qs * P : (qs + 1) * P],
                                v_t[kc][:],
                                start=(kc == 0),
                                stop=(kc == nkc - 1),
                            )
                        if use_den:
                            nc.vector.tensor_scalar_mul(
                                ob[:], op[:], recip_sb[:, qs : qs + 1]
                            )
                        else:
                            nc.vector.tensor_copy(ob[:], op[:])
                    else:
                        nc.vector.tensor_copy(ob[:], at_tiles[qs][:, 0:dv])
                    nc.sync.dma_start(
                        od[q0 + qs * P : q0 + (qs + 1) * P, :], ob[:]
                    )


_CACHE = {}


def _get_compiled():
    if "nc" not in _CACHE:
        nc = bacc.Bacc(
            "TRN2", target_bir_lowering=False, debug=False, num_devices=B
        )
        build_attention(nc)
        nc.compile()
        _CACHE["nc"] = nc
    return _CACHE["nc"]


def make_mask_bias(mask_out):
    """[B, 1, S] bool -> [B, P, S//P] f32 additive bias in chunk layout."""
    m = np.asarray(mask_out).reshape(B, S)
    mb = np.where(m, np.float32(MASK_BIAS), np.float32(0.0)).astype(np.float32)
    return np.ascontiguousarray(mb.reshape(B, S // P, P).transpose(0, 2, 1))


def run(Q, K, V, mask_out, **spmd_kwargs):
    """Returns (full_output, BassKernelResults)."""
    Q = np.asarray(Q, dtype=np.float32)
    K = np.asarray(K, dtype=np.float32)
    V = np.asarray(V, dtype=np.float32)
    mb = make_mask_bias(mask_out)

    nc = _get_compiled()
    in_maps = [
        {
            "Q": np.ascontiguousarray(Q[b]),
            "K": np.ascontiguousarray(K[b]),
            "V": np.ascontiguousarray(V[b]),
            "MB": mb[b],
        }
        for b in range(B)
    ]
    res = run_bass_kernel_spmd(nc, in_maps, list(range(B)), **spmd_kwargs)
    out = np.stack([res.results[b]["out"] for b in range(B)]).astype(np.float32)
    return out, res


def kernel(Q, K, V, mask_out):
    return run(Q, K, V, mask_out)[0]


# revision 34
# speedup vs baseline: 1.2219x; 1.2219x over previous
"""Dot-product attention (B=8, S=2048, DK=DV=512) on 8 TRN2 NeuronCores.

Data-parallel: one batch element per core. Per core, in transposed-score
layout (so the exp'ed attention chunks are directly the stationary operand
of the second matmul — no attention transposes needed):

    scores^T[k, q] = K @ Q^T / sqrt(DK)      (PE, fp32r, 1 col/cycle)
    attn^T = exp(scores^T + mask_bias[k])    (ACT, fused scale+bias+exp)
    den[q] = sum_k attn^T[k, q]              (DVE chunk-accumulate ->
                                              PE transpose -> DVE reduce;
                                              NO tiny-dim matmuls: M=1/N=1
                                              matmuls cost ~10us each on HW)
    out[q, v] = (attn^T).T @ V / den[q]      (PE accumulate + DVE normalize)

Only Q and K need physical transposes ([d, seq] layout for the contraction);
done on the PE via identity-matmul transpose, pipelined with the DMA loads.
scores are ~N(0,1) so softmax without max-subtraction is numerically safe;
masked keys get an additive -1e4 bias folded into the exp (exp -> 0).
"""

import os
from contextlib import ExitStack

import numpy as np

import concourse.mybir as mybir
import concourse.tile as tile
from concourse import bacc
from concourse.bass_utils import run_bass_kernel_spmd
from concourse.masks import make_identity

B = 8
S = 2048
DK = 512
DV = 512
P = 128

F32 = mybir.dt.float32
MASK_BIAS = -10000.0

# matmul compute dtype: float32r streams 1 row/cycle (vs 4 for float32) with
# fp32 storage and relaxed multiply precision; producers must round to it.
# Set BASS_ATTN_MM_DT=float32 (exact, 4x slower) or bfloat16 to change.
MM_DT = getattr(mybir.dt, os.environ.get("BASS_ATTN_MM_DT", "float32r"))


def build_attention(
    nc,
    s=S,
    dk=DK,
    dv=DV,
    q_tile=512,
    n_reps=1,
    use_t=True,
    use_exp=True,
    use_den=True,
    use_mm2=True,
):
    """Emit the per-core attention kernel into `nc` (TileContext inside).

    n_reps > 1 repeats the whole computation (for benchmarking: one NEFF
    timing K serialized executions; output is overwritten each rep).
    use_* flags disable pieces for timing bisection (output garbage).
    """
    scale = 1.0 / float(np.sqrt(dk))
    nkc = s // P          # key chunks (partition dim of scores^T)
    ndc = dk // P         # contraction chunks for QK^T
    nqt = s // q_tile     # outer q tiles
    nqs = q_tile // P     # q subtiles per q tile
    nst = s // P          # seq tiles for the Q/K transpose prep

    qd = nc.declare_dram_parameter("Q", [s, dk], F32, isOutput=False).ap()
    kd = nc.declare_dram_parameter("K", [s, dk], F32, isOutput=False).ap()
    vd = nc.declare_dram_parameter("V", [s, dv], F32, isOutput=False).ap()
    mbd = nc.declare_dram_parameter("MB", [P, nkc], F32, isOutput=False).ap()
    od = nc.declare_dram_parameter("out", [s, dv], F32, isOutput=True).ap()

    with ExitStack() as ctx:
        tc = ctx.enter_context(tile.TileContext(nc))
        constp = ctx.enter_context(tc.tile_pool(name="const", bufs=1))
        qtp = ctx.enter_context(tc.tile_pool(name="qtp", bufs=nqt + 1))
        ktp = ctx.enter_context(tc.tile_pool(name="ktp", bufs=nst + 2))
        vp = ctx.enter_context(tc.tile_pool(name="vp", bufs=nkc + 2))
        stagep = ctx.enter_context(tc.tile_pool(name="stage", bufs=8))
        attp = ctx.enter_context(tc.tile_pool(name="att", bufs=nkc + 4))
        osbp = ctx.enter_context(tc.tile_pool(name="osb", bufs=3))
        dsbp = ctx.enter_context(tc.tile_pool(name="dsb", bufs=2))
        tpp = ctx.enter_context(tc.tile_pool(name="tpp", bufs=2, space="PSUM"))
        stp = ctx.enter_context(tc.tile_pool(name="stp", bufs=2, space="PSUM"))
        opp = ctx.enter_context(tc.tile_pool(name="opp", bufs=2, space="PSUM"))
        dentp = ctx.enter_context(tc.tile_pool(name="dentp", bufs=2, space="PSUM"))

        id_sb = constp.tile([P, P], F32)
        make_identity(nc, id_sb[:])
        id_r = None
        if MM_DT == mybir.dt.float32r:
            id_r = constp.tile([P, P], MM_DT)
            nc.vector.tensor_copy(id_r[:], id_sb[:])
        mb_sb = constp.tile([P, nkc], F32)
        nc.sync.dma_start(mb_sb[:], mbd[:, :])

        # warm-up exp on a scratch tile: pulls the ~2.7us ACT exp-table load
        # off the critical path (it overlaps the phase-1 DMAs instead of
        # stalling the first score chunk)
        act_warm = constp.tile([P, 1], F32)
        nc.scalar.activation(
            act_warm[:], id_sb[:, 0:1], mybir.ActivationFunctionType.Exp
        )

        for _rep in range(n_reps):
            # Transposed operands, fine-grained so DMA/transpose pipeline
            # into the matmuls:
            #   qt_q[i]  = Q^T for q tile i   [P, ndc, q_tile]
            #   kt_t[t]  = K^T for key chunk t [P, ndc, P]
            #   v_t[kc]  = natural V chunk     [P, dv]
            qt_q = [qtp.tile([P, ndc, q_tile], MM_DT, name=f"qtq{i}", tag="qtq")
                    for i in range(nqt)]
            kt_t = [ktp.tile([P, ndc, P], MM_DT, name=f"ktt{t}", tag="ktt")
                    for t in range(nst)]
            v_t = [vp.tile([P, dv], MM_DT, name=f"vt{kc}", tag="vt")
                   for kc in range(nkc)]

            tr_dt = (
                MM_DT
                if MM_DT == mybir.dt.float32r
                and os.environ.get("BASS_ATTN_TR_DT", "f32") == "f32r"
                else F32
            )

            def load_transpose(src, t, dst):
                nat = stagep.tile([P, dk], F32, tag="nat", name="nat")
                nc.sync.dma_start(nat[:], src[t * P : (t + 1) * P, :])
                if tr_dt == F32:
                    natr = nat
                else:
                    # round to fp32r on DVE so the PE transpose (a matmul)
                    # runs at 1.5 cycles/row instead of f32's 2.0
                    natr = stagep.tile([P, dk], tr_dt, tag="natr", name="natr")
                    nc.vector.tensor_copy(natr[:], nat[:])
                tp = tpp.tile([P, ndc * P], tr_dt, name="tp", tag="tp")
                for dc in range(ndc):
                    nc.tensor.transpose(
                        tp[:, dc * P : (dc + 1) * P],
                        natr[:, dc * P : (dc + 1) * P],
                        id_sb[:] if tr_dt == F32 else id_r[:],
                    )
                nc.vector.tensor_copy(
                    dst, tp[:].rearrange("p (dc q) -> p dc q", dc=ndc)
                )

            def load_v(kc):
                if MM_DT == F32:
                    nc.sync.dma_start(v_t[kc][:], vd[kc * P : (kc + 1) * P, :])
                else:
                    vn = stagep.tile([P, dv], F32, tag="vn", name="vn")
                    nc.sync.dma_start(vn[:], vd[kc * P : (kc + 1) * P, :])
                    nc.vector.tensor_copy(v_t[kc][:], vn[:])

            # ---- phase 1: emit in consumption order: K chunk 0-3 and Q
            # tile 0 first (unblocks the first matmuls), then the rest.
            if use_t:
                sq = q_tile // P
                for t in range(sq):
                    load_transpose(kd, t, kt_t[t][:, :, :])
                for t in range(sq):
                    load_transpose(qd, t, qt_q[0][:, :, t * P : (t + 1) * P])
                for t in range(sq, nst):
                    load_transpose(kd, t, kt_t[t][:, :, :])
                    load_transpose(
                        qd, t, qt_q[t // sq][:, :, (t % sq) * P : (t % sq + 1) * P]
                    )
            for kc in range(nkc):
                load_v(kc)

            # ---- phase 2: attention over q tiles ----
            for qt_i in range(nqt):
                q0 = qt_i * q_tile
                at_tiles = []
                acc = dsbp.tile([P, q_tile], F32, tag="acc")
                for kc in range(nkc):
                    st = stp.tile([P, q_tile], F32)
                    for dc in range(ndc):
                        nc.tensor.matmul(
                            st[:],
                            kt_t[kc][:, dc, :],
                            qt_q[qt_i][:, dc, :],
                            start=(dc == 0),
                            stop=(dc == ndc - 1),
                        )
                    at = attp.tile([P, q_tile], MM_DT, tag="at")
                    nc.scalar.activation(
                        at[:],
                        st[:],
                        mybir.ActivationFunctionType.Exp
                        if use_exp
                        else mybir.ActivationFunctionType.Copy,
                        bias=mb_sb[:, kc : kc + 1] if use_exp else 0.0,
                        scale=scale,
                    )
                    at_tiles.append(at)
                    if use_den:
                        # running sum of attn chunks over key chunks (DVE);
                        # fp32r holds f32 bits, read it as f32 for the add
                        atf = (
                            at[:].bitcast(F32)
                            if MM_DT == mybir.dt.float32r
                            else at[:]
                        )
                        if kc == 0:
                            nc.vector.tensor_copy(acc[:], atf)
                        else:
                            nc.vector.tensor_add(acc[:], acc[:], atf)

                # den[q] = partition-sum of acc: PE-transpose the 4 q-slices
                # into one PSUM bank, one 3D free-dim reduce, one reciprocal.
                if use_den:
                    dent_ps = dentp.tile([P, nqs * P], F32)
                    for qs in range(nqs):
                        nc.tensor.transpose(
                            dent_ps[:, qs * P : (qs + 1) * P],
                            acc[:, qs * P : (qs + 1) * P],
                            id_sb[:],
                        )
                    den_sb = dsbp.tile([P, nqs], F32, tag="den")
                    nc.vector.reduce_sum(
                        den_sb[:],
                        dent_ps[:].rearrange("p (qs k) -> p qs k", qs=nqs),
                        axis=mybir.AxisListType.X,
                    )
                    recip_sb = dsbp.tile([P, nqs], F32, tag="recip")
                    nc.vector.reciprocal(recip_sb[:], den_sb[:])

                for qs in range(nqs):
                    ob = osbp.tile([P, dv], F32, tag="ob")
                    if use_mm2:
                        op = opp.tile([P, dv], F32)
                        for kc in range(nkc):
                            nc.tensor.matmul(
                                op[:],
                                at_tiles[kc][:, qs * P : (qs + 1) * P],
                                v_t[kc][:],
                                start=(kc == 0),
                                stop=(kc == nkc - 1),
                            )
                        if use_den:
                            nc.vector.tensor_scalar_mul(
                                ob[:], op[:], recip_sb[:, qs : qs + 1]
                            )
                        else:
                            nc.vector.tensor_copy(ob[:], op[:])
                    else:
                        nc.vector.tensor_copy(ob[:], at_tiles[qs][:, 0:dv])
                    nc.sync.dma_start(
                        od[q0 + qs * P : q0 + (qs + 1) * P, :], ob[:]
                    )


_CACHE = {}


def _get_compiled():
    if "nc" not in _CACHE:
        nc = bacc.Bacc(
            "TRN2", target_bir_lowering=False, debug=False, num_devices=B
        )
        build_attention(nc)
        nc.compile()
        _CACHE["nc"] = nc
    return _CACHE["nc"]


def make_mask_bias(mask_out):
    """[B, 1, S] bool -> [B, P, S//P] f32 additive bias in chunk layout."""
    m = np.asarray(mask_out).reshape(B, S)
    mb = np.where(m, np.float32(MASK_BIAS), np.float32(0.0)).astype(np.float32)
    return np.ascontiguousarray(mb.reshape(B, S // P, P).transpose(0, 2, 1))


def run(Q, K, V, mask_out, **spmd_kwargs):
    """Returns (full_output, BassKernelResults)."""
    Q = np.asarray(Q, dtype=np.float32)
    K = np.asarray(K, dtype=np.float32)
    V = np.asarray(V, dtype=np.float32)
    mb = make_mask_bias(mask_out)

    nc = _get_compiled()
    in_maps = [
        {
            "Q": np.ascontiguousarray(Q[b]),
            "K": np.ascontiguousarray(K[b]),
            "V": np.ascontiguousarray(V[b]),
            "MB": mb[b],
        }
        for b in range(B)
    ]
    res = run_bass_kernel_spmd(nc, in_maps, list(range(B)), **spmd_kwargs)
    out = np.stack([res.results[b]["out"] for b in range(B)]).astype(np.float32)
    return out, res


def kernel(Q, K, V, mask_out):
    return run(Q, K, V, mask_out)[0]
